# revision 8
# baseline (speedup 1.0000x reference)
"""Trainium2 Bass kernel for ChannelFeatures (channel-attention style module).

Computes, per batch element b:
    x_max[b] = max over (H,W) of features[b]          # (C,)
    x_avg[b] = mean over (H,W) of features[b]         # (C,)
    7 residual blocks (shared weights on both branches):
        x = prelu(W1[k] @ x + b1[k], a1[k]) + x
    scores[b] = sigmoid(x_max[b] + x_avg[b])          # (C,)
    out[b] = features[b] * scores[b]                  # broadcast over (H,W)

Sharding: pure data parallel over batch — 16 batch elements across 8 cores,
2 per core, weights replicated. No cross-core communication.

Device strategy per core (2 batch elements, each (65536, 64) fp32):
  The kernel is HBM-bound: 33.5 MB in + 33.5 MB out per core, and the HBM
  streams run at ~425 GB/s per direction. Every byte moves exactly once:

  * Loads are SWDGE cast-DMAs (fp32 DRAM -> fp16 SBUF), 2 MB (two tiles) per
    DMA so the SWDGE ring's in-flight credit never paces the stream; the
    whole 32 MB working set stays resident as 16 MB of fp16.
  * Max: tensor_tensor max trees per pair (16-bit 2x DVE mode) folded into a
    running (P, 4, C) max, so the per-batch serial tail is ~1 us.
  * Sum: PE ones-matmuls off the fp16 tiles, PSUM-accumulated per batch
    (mean scale folded into the row->column transpose matmul).
  * Scores broadcast to 128 partitions via a K=1 ones matmul on PE.
  * Pass 2: in-place fp16 multiply (2x mode), ACT widens to fp32 staging,
    stores go out on the HWDGE (SP) ring so the write stream overlaps the
    next batch's SWDGE read stream at the SDMA engines.
  * tile_wait_until hints encode the MEASURED load cadence and score-chain
    latency: Tile's cost model thinks loads are ~2.4x faster than reality
    and would otherwise order batch b+1's reduce ops ahead of batch b's
    multiplies on DVE (head-of-line-blocking the store stream) — the hints
    interleave them the way the hardware actually runs.
"""

import numpy as np
from contextlib import ExitStack, nullcontext

import concourse.bass as bass
import concourse.tile as tile
from concourse import masks, mybir
from concourse.bass_utils import run_bass_kernel_spmd

# Problem shapes (hardcoded per contract)
B, H, W, C = 16, 256, 256, 64
CONV_NUM = 7
NCORES = 8
BPC = B // NCORES          # batch elements per core
HW = H * W                 # 65536 spatial positions
P = 128                    # SBUF partitions
KF = 32                    # spatial rows per partition per tile
TILE_ROWS = P * KF         # 4096 spatial rows per tile
T = HW // TILE_ROWS        # 16 tiles per batch element
QP = 2                     # tiles per load DMA
NQ = T // QP               # pair-loads per batch element
F32 = mybir.dt.float32
F16 = mybir.dt.float16     # fp16: 16-bit DVE/PE fast paths, 4x bf16 mantissa

# Scheduler model hints (us), from measured HW traces: load pairs land every
# ~4.7 us; the final-tree + recurrence + scores chain is ~16 us; muls pace
# ~1.3 us apart on DVE.
LOAD0_US = 10.0
LOADP_US = 4.72
CHAIN_US = 16.0
MUL_US = 1.3

# test.py hooks: set PROFILE=True before calling kernel() to capture an NTFF
# trace; LAST_EXEC_NS then holds the max per-core HW execution time.
PROFILE = False
LAST_EXEC_NS = None
LAST_RESULTS = None


def _split_dma_waits(nc: bass.Bass) -> None:
    """The pinned walrus build rejects DMA instructions carrying more than one
    sync-wait ("Too many sync wait commands"). Tile's sem assignment is not
    transitively minimal, so slot-reuse instructions can get two waits
    (consumer release + WAW with the previous writer). Hoist all but the last
    wait onto wait-only EventSemaphore instructions on the same engine right
    before the instruction."""
    n = 0
    # num=200: outside every id Tile allocated (its end-of-kernel range-clear
    # covers the allocated block), so no collision with released Tile sems.
    dummy = nc.alloc_semaphore(name="wsplit_dummy", num=200)
    for fn in nc.m.functions:
        for blk in fn.blocks:
            new_insts = []
            for inst in blk.instructions:
                si = getattr(inst, "sync_info", None)
                if si is not None and len(si.on_wait) > 1:
                    for w in si.on_wait[:-1]:
                        ev = mybir.InstEventSemaphore(
                            name=f"WSPLIT-{n}", ins=[], outs=[]
                        )
                        n += 1
                        ev.engine = inst.engine
                        # Tick a dedicated dummy sem nobody waits on, so the
                        # simulator/race tooling (which require every
                        # instruction to carry an update) accept the carrier.
                        upd = mybir.SyncUpdate(
                            sync_type="semaphore",
                            id=dummy.num,
                            ant_name=dummy.name,
                            update_mode="sem-add-imm",
                            update_value=1,
                        )
                        ev.sync_info = mybir.SyncInfo(on_wait=[w], on_update=[upd])
                        new_insts.append(ev)
                    si.on_wait = [si.on_wait[-1]]
                new_insts.append(inst)
            blk.instructions = new_insts


def _build_nc() -> bass.Bass:
    nc = bass.Bass()
    feat = nc.declare_dram_parameter("features", [BPC, HW, C], F32, isOutput=False)
    wT = nc.declare_dram_parameter("wT", [C, CONV_NUM, C], F32, isOutput=False)
    bT = nc.declare_dram_parameter("bT", [C, CONV_NUM], F32, isOutput=False)
    aT = nc.declare_dram_parameter("aT", [C, CONV_NUM], F32, isOutput=False)
    out = nc.declare_dram_parameter("out", [BPC, HW, C], F32, isOutput=True)

    # pair-load view: dest[p, u, k, c] <- row (2q+u)*4096 + p*32 + k
    feat_p = feat[:].rearrange("b (q u p k) c -> b q p u k c", u=QP, p=P, k=KF)
    out_t = out[:].rearrange("b (t p k) c -> b t p k c", p=P, k=KF)

    SEG = KF // 8            # 512-wide matmul segments per tile
    MAX = mybir.AluOpType.max

    with ExitStack() as ctx:
        tc = ctx.enter_context(tile.TileContext(nc))
        singles = ctx.enter_context(tc.tile_pool(name="singles", bufs=1))
        cache = ctx.enter_context(tc.tile_pool(name="cache", bufs=1))
        stgp = ctx.enter_context(tc.tile_pool(name="stgp", bufs=4))
        treep = ctx.enter_context(tc.tile_pool(name="treep", bufs=2))
        runp = ctx.enter_context(tc.tile_pool(name="runp", bufs=2))
        sctp = ctx.enter_context(tc.tile_pool(name="sctp", bufs=2))
        small = ctx.enter_context(tc.tile_pool(name="small", bufs=2))
        psum = ctx.enter_context(tc.tile_pool(name="psum", bufs=1, space="PSUM"))
        psum2 = ctx.enter_context(tc.tile_pool(name="psum2", bufs=2, space="PSUM"))

        # Constants (HWDGE loads; the SWDGE/POOL queue stays clear for tiles)
        w_sb = singles.tile([C, CONV_NUM, C], F32)   # [c_in, k, c_out]
        nc.sync.dma_start(out=w_sb[:], in_=wT[:])
        b_sb = singles.tile([C, CONV_NUM], F32)      # [c, k]
        nc.sync.dma_start(out=b_sb[:], in_=bT[:])
        a_sb = singles.tile([C, CONV_NUM], F32)      # [c, k] (a1[k] per row)
        nc.sync.dma_start(out=a_sb[:], in_=aT[:])
        ones_col = singles.tile([P, 1], F16)
        nc.vector.memset(ones_col[:], 1.0)
        ones_row = singles.tile([1, P], F32)
        nc.vector.memset(ones_row[:], 1.0)
        one_hw = singles.tile([1, 1], F32)
        nc.vector.memset(one_hw[:], 1.0 / HW)
        identity = singles.tile([P, P], F32)

        # [channel, branch(0=max,1=avg), batch]
        xvec = singles.tile([C, 2, BPC], F32)

        for b in range(BPC):
            # ---- Pass 1(b): cast-load tile pairs, reduce as they land ----
            pairs = []
            run = runp.tile([P, 4, C], F16, tag="run")
            psum_s = psum2.tile([1, 8 * C], F32, tag="psum_s")
            for q in range(NQ):
                pt = cache.tile([P, QP, KF, C], F16, tag=f"c{b}_{q}")
                nc.gpsimd.dma_start(out=pt[:], in_=feat_p[b, q])
                pairs.append(pt)
                tw = LOAD0_US + LOADP_US * (NQ * b + q + 1)
                with tc.tile_wait_until(tw / 1000):
                    # per-pair max tree (2x 16-bit TT mode) into a running max
                    l1 = treep.tile([P, QP, 16, C], F16, tag="tree")
                    nc.vector.tensor_tensor(
                        l1[:], pt[:, :, :16], pt[:, :, 16:], MAX
                    )
                    nc.vector.tensor_tensor(
                        l1[:, :, :8], l1[:, :, :8], l1[:, :, 8:], MAX
                    )
                    nc.vector.tensor_tensor(
                        l1[:, :, :4], l1[:, :, :4], l1[:, :, 4:8], MAX
                    )
                    nc.vector.tensor_tensor(
                        l1[:, 0, :4], l1[:, 0, :4], l1[:, 1, :4], MAX
                    )
                    if q == 0:
                        nc.vector.tensor_copy(run[:], l1[:, 0, :4])
                    else:
                        nc.vector.tensor_tensor(run[:], run[:], l1[:, 0, :4], MAX)
                    # sum: PE ones-matmuls, PSUM-accumulated over the batch;
                    # the (row, channel) mix is folded at the end.
                    sv = pt[:].rearrange("p u (s r) c -> p (u s) (r c)", s=SEG)
                    for seg in range(QP * SEG):
                        nc.tensor.matmul(
                            psum_s[:],
                            ones_col[:],
                            sv[:, seg],
                            start=(q == 0 and seg == 0),
                            stop=(q == NQ - 1 and seg == QP * SEG - 1),
                        )
            if b == 0:
                # after the b0 load triggers are queued so it doesn't delay
                # them (make_identity runs on the gpsimd engine)
                masks.make_identity(nc, identity[:])

            # short final tree 4 -> 2 -> 1, then cross-partition via PE
            # transpose + DVE reduce
            s2t = small.tile([P, 2, C], F16, tag="s2")
            nc.vector.tensor_tensor(s2t[:], run[:, :2], run[:, 2:], MAX)
            maxr = small.tile([P, C], F32, tag="maxr")
            nc.vector.tensor_tensor(maxr[:], s2t[:, 0], s2t[:, 1], MAX)
            mt = psum.tile([C, P], F32, tag="mt")
            nc.tensor.transpose(mt[:], maxr[:], identity[:])
            nc.vector.reduce_max(
                out=xvec[:, 0, b : b + 1], in_=mt[:], axis=mybir.AxisListType.X
            )
            # fold (row, channel) mix: (1, C, 8) reduce -> (1, C)
            srow = small.tile([1, C], F32, tag="srow")
            nc.vector.reduce_sum(
                out=srow[:],
                in_=psum_s[:].rearrange("p (s c) -> p c s", c=C),
                axis=mybir.AxisListType.X,
            )
            # transpose row->column via K=1 matmul, folding the 1/HW scale
            av = psum.tile([C, 1], F32, tag="av")
            nc.tensor.matmul(av[:], srow[:], one_hw[:], start=True, stop=True)
            nc.vector.tensor_copy(xvec[:, 1, b : b + 1], av[:])

            # ---- Recurrence(b): 7 residual PReLU blocks on (C, 2) ----
            # 1 PE matmul + 4 DVE ops per block (bias folded into the
            # tensor_scalar ops, a*neg+pos fused via scalar_tensor_tensor).
            xf = xvec[:, :, b]  # (C, 2): cols = (max, avg)
            for k in range(CONV_NUM):
                y = psum.tile([C, 2], F32, tag="y")
                nc.tensor.matmul(y[:], w_sb[:, k, :], xf, start=True, stop=True)
                pos = small.tile([C, 2], F32, tag="pos")
                nc.vector.tensor_scalar(
                    pos[:], y[:], b_sb[:, k : k + 1], 0.0,
                    mybir.AluOpType.add, mybir.AluOpType.max,
                )
                zmin = small.tile([C, 2], F32, tag="zmin")
                nc.vector.tensor_scalar(
                    zmin[:], y[:], b_sb[:, k : k + 1], 0.0,
                    mybir.AluOpType.add, mybir.AluOpType.min,
                )
                pn = small.tile([C, 2], F32, tag="pn")
                nc.vector.scalar_tensor_tensor(
                    pn[:], zmin[:], a_sb[:, k : k + 1], pos[:],
                    mybir.AluOpType.mult, mybir.AluOpType.add,
                )
                xn = small.tile([C, 2], F32, tag="xn")
                nc.vector.tensor_add(xn[:], pn[:], xf)
                xf = xn[:]

            # scores(b) = sigmoid(x_max + x_avg): (C, 1)
            ssum = small.tile([C, 1], F32, tag="ssum")
            nc.vector.tensor_add(ssum[:], xf[:, 0:1], xf[:, 1:2])
            scores = small.tile([C, 1], F32, tag="scores")
            nc.scalar.activation(
                out=scores[:], in_=ssum[:], func=mybir.ActivationFunctionType.Sigmoid
            )
            # broadcast to all partitions on-chip: (C,1) -T-> (1,C), then a
            # K=1 ones matmul fans it out to (P, C); widen to a full fp16
            # (P, KF, C) tile so the pass-2 muls see dense step-1 operands.
            sc_t = psum.tile([1, C], F32, tag="sc_t")
            nc.tensor.transpose(sc_t[:], scores[:], identity[:C, :C])
            sc_sb = small.tile([1, C], F32, tag="sc_sb")
            nc.vector.tensor_copy(sc_sb[:], sc_t[:])
            bc_ps = psum.tile([P, C], F32, tag="bc")
            nc.tensor.matmul(bc_ps[:], ones_row[:], sc_sb[:], start=True, stop=True)
            bcb = small.tile([P, C], F16, tag="bcb")
            nc.vector.tensor_copy(bcb[:], bc_ps[:])
            sct = sctp.tile([P, KF, C], F16, tag="sct")
            nc.vector.tensor_copy(
                sct[:], bcb[:].unsqueeze(1).to_broadcast([P, KF, C])
            )

            # ---- Pass 2(b): 16-bit multiply in place, ACT widens to fp32
            # staging, stores ride the HWDGE (SP) ring so the write stream
            # overlaps the next batch's SWDGE read stream.
            for t in range(T):
                q, u = divmod(t, QP)
                src = pairs[q][:, u]
                mw = LOAD0_US + LOADP_US * NQ * (b + 1) + CHAIN_US + MUL_US * t
                with tc.tile_wait_until(mw / 1000):
                    nc.vector.tensor_mul(src, src, sct[:])
                stg = stgp.tile([P, KF, C], F32, tag="stg")
                nc.scalar.copy(out=stg[:], in_=src)
                nc.sync.dma_start(out=out_t[b, t], in_=stg[:])

    _split_dma_waits(nc)
    return nc


def _prep_inputs(features, W1, b1, a1):
    feats = np.ascontiguousarray(features, dtype=np.float32).reshape(B, HW, C)
    # lhsT layout: wT[c_in, k, c_out] = W1[k, c_out, c_in]
    wT = np.ascontiguousarray(np.transpose(np.asarray(W1, np.float32), (2, 0, 1)))
    bT = np.ascontiguousarray(np.asarray(b1, np.float32).T)            # (C, 7)
    aT = np.ascontiguousarray(
        np.broadcast_to(np.asarray(a1, np.float32), (C, CONV_NUM))
    )
    return feats, wT, bT, aT


def kernel(features, W1, b1, a1):
    global LAST_EXEC_NS
    feats, wT, bT, aT = _prep_inputs(features, W1, b1, a1)
    nc = _build_nc()
    in_maps = [
        {
            "features": feats[i * BPC : (i + 1) * BPC],
            "wT": wT,
            "bT": bT,
            "aT": aT,
        }
        for i in range(NCORES)
    ]
    import os

    res = run_bass_kernel_spmd(
        nc,
        in_maps,
        list(range(NCORES)),
        trace=PROFILE,
        tmpdir=os.environ.get("BASS_TMPDIR"),
    )
    global LAST_RESULTS
    LAST_RESULTS = res
    LAST_EXEC_NS = res.exec_time_ns
    out = np.concatenate(
        [res.results[i]["out"].reshape(BPC, H, W, C) for i in range(NCORES)], axis=0
    )
    return out


# revision 9
# speedup vs baseline: 1.0164x; 1.0164x over previous
"""Trainium2 Bass kernel for ChannelFeatures (channel-attention style module).

Computes, per batch element b:
    x_max[b] = max over (H,W) of features[b]          # (C,)
    x_avg[b] = mean over (H,W) of features[b]         # (C,)
    7 residual blocks (shared weights on both branches):
        x = prelu(W1[k] @ x + b1[k], a1[k]) + x
    scores[b] = sigmoid(x_max[b] + x_avg[b])          # (C,)
    out[b] = features[b] * scores[b]                  # broadcast over (H,W)

Sharding: pure data parallel over batch — 16 batch elements across 8 cores,
2 per core, weights replicated. No cross-core communication.

Device strategy per core (2 batch elements, each (65536, 64) fp32):
  The kernel is HBM-bound: 33.5 MB in + 33.5 MB out per core, and the two
  HBM directions each sustain ~425 GB/s with measurable independence, so the
  structure keeps the read stream (SWDGE ring) and write stream (HWDGE SP
  ring) concurrently busy:

  * Loads: 32x 1 MB SWDGE cast-DMAs (fp32 DRAM -> fp16 SBUF) — 1 MB is the
    sweet spot (the SWDGE descriptor-emission rate limits bigger chunked
    transfers); the whole 32 MB working set stays resident as 16 MB fp16.
  * Max: per-tile tensor_tensor max trees (16-bit 2x DVE mode) folded into a
    running (P, 4, C) max — the per-batch serial tail is well under 1 us.
  * Sum: PE ones-matmuls off the fp16 tiles, PSUM-accumulated per batch
    (mean scale folded into the row->column transpose matmul).
  * Recurrence: 1 PE matmul + 4 DVE ops per block (bias folded into
    tensor_scalar, a*neg+pos fused via scalar_tensor_tensor).
  * Scores broadcast to 128 partitions via a K=1 ones matmul on PE.
  * Pass 2: in-place fp16 multiply (2x mode) -> ACT widens into a 5-deep
    fp32 staging pool -> HWDGE stores. Batch 0's multiplies are ordered
    AHEAD of batch 1's tree ops on DVE (tile_wait_until pushes the latter
    back in the scheduler's model clock, which otherwise trusts its
    2.4x-too-fast DMA model), so the write stream starts as soon as batch
    0's scores exist and overlaps batch 1's read stream.
"""

import numpy as np
from contextlib import ExitStack, nullcontext

import concourse.bass as bass
import concourse.tile as tile
from concourse import masks, mybir
from concourse.bass_utils import run_bass_kernel_spmd

# Problem shapes (hardcoded per contract)
B, H, W, C = 16, 256, 256, 64
CONV_NUM = 7
NCORES = 8
BPC = B // NCORES          # batch elements per core
HW = H * W                 # 65536 spatial positions
P = 128                    # SBUF partitions
KF = 32                    # spatial rows per partition per tile
TILE_ROWS = P * KF         # 4096 spatial rows per tile
T = HW // TILE_ROWS        # 16 tiles per batch element
F32 = mybir.dt.float32
F16 = mybir.dt.float16     # fp16: 16-bit DVE/PE fast paths, 4x bf16 mantissa

# test.py hooks: set PROFILE=True before calling kernel() to capture an NTFF
# trace; LAST_EXEC_NS then holds the max per-core HW execution time.
PROFILE = False
LAST_EXEC_NS = None
LAST_RESULTS = None


def _split_dma_waits(nc: bass.Bass) -> None:
    """The pinned walrus build rejects DMA instructions carrying more than one
    sync-wait ("Too many sync wait commands"). Tile's sem assignment is not
    transitively minimal, so slot-reuse instructions can get two waits
    (consumer release + WAW with the previous writer). Hoist all but the last
    wait onto wait-only EventSemaphore instructions on the same engine right
    before the instruction."""
    n = 0
    # num=200: outside every id Tile allocated (its end-of-kernel range-clear
    # covers the allocated block), so no collision with released Tile sems.
    dummy = nc.alloc_semaphore(name="wsplit_dummy", num=200)
    for fn in nc.m.functions:
        for blk in fn.blocks:
            new_insts = []
            for inst in blk.instructions:
                si = getattr(inst, "sync_info", None)
                if si is not None and len(si.on_wait) > 1:
                    for w in si.on_wait[:-1]:
                        ev = mybir.InstEventSemaphore(
                            name=f"WSPLIT-{n}", ins=[], outs=[]
                        )
                        n += 1
                        ev.engine = inst.engine
                        # Tick a dedicated dummy sem nobody waits on, so the
                        # simulator/race tooling (which require every
                        # instruction to carry an update) accept the carrier.
                        upd = mybir.SyncUpdate(
                            sync_type="semaphore",
                            id=dummy.num,
                            ant_name=dummy.name,
                            update_mode="sem-add-imm",
                            update_value=1,
                        )
                        ev.sync_info = mybir.SyncInfo(on_wait=[w], on_update=[upd])
                        new_insts.append(ev)
                    si.on_wait = [si.on_wait[-1]]
                new_insts.append(inst)
            blk.instructions = new_insts


def _build_nc() -> bass.Bass:
    nc = bass.Bass()
    feat = nc.declare_dram_parameter("features", [BPC, HW, C], F32, isOutput=False)
    wT = nc.declare_dram_parameter("wT", [C, CONV_NUM, C], F32, isOutput=False)
    bT = nc.declare_dram_parameter("bT", [C, CONV_NUM], F32, isOutput=False)
    aT = nc.declare_dram_parameter("aT", [C, CONV_NUM], F32, isOutput=False)
    out = nc.declare_dram_parameter("out", [BPC, HW, C], F32, isOutput=True)

    feat_t = feat[:].rearrange("b (t p k) c -> b t p k c", p=P, k=KF)
    out_t = out[:].rearrange("b (t p k) c -> b t p k c", p=P, k=KF)

    SEG = KF // 8            # 512-wide matmul segments per tile
    MAX = mybir.AluOpType.max

    with ExitStack() as ctx:
        tc = ctx.enter_context(tile.TileContext(nc))
        singles = ctx.enter_context(tc.tile_pool(name="singles", bufs=1))
        cache = ctx.enter_context(tc.tile_pool(name="cache", bufs=1))
        stgp = ctx.enter_context(tc.tile_pool(name="stgp", bufs=5))
        treep = ctx.enter_context(tc.tile_pool(name="treep", bufs=2))
        runp = ctx.enter_context(tc.tile_pool(name="runp", bufs=2))
        sctp = ctx.enter_context(tc.tile_pool(name="sctp", bufs=2))
        small = ctx.enter_context(tc.tile_pool(name="small", bufs=2))
        psum = ctx.enter_context(tc.tile_pool(name="psum", bufs=1, space="PSUM"))
        psum2 = ctx.enter_context(tc.tile_pool(name="psum2", bufs=2, space="PSUM"))

        # Constants (HWDGE loads; the SWDGE/POOL queue stays clear for tiles)
        w_sb = singles.tile([C, CONV_NUM, C], F32)   # [c_in, k, c_out]
        nc.sync.dma_start(out=w_sb[:], in_=wT[:])
        b_sb = singles.tile([C, CONV_NUM], F32)      # [c, k]
        nc.sync.dma_start(out=b_sb[:], in_=bT[:])
        a_sb = singles.tile([C, CONV_NUM], F32)      # [c, k] (a1[k] per row)
        nc.sync.dma_start(out=a_sb[:], in_=aT[:])
        ones_col = singles.tile([P, 1], F16)
        nc.vector.memset(ones_col[:], 1.0)
        ones_row = singles.tile([1, P], F32)
        nc.vector.memset(ones_row[:], 1.0)
        one_hw = singles.tile([1, 1], F32)
        nc.vector.memset(one_hw[:], 1.0 / HW)
        identity = singles.tile([P, P], F32)

        # [channel, branch(0=max,1=avg), batch]
        xvec = singles.tile([C, 2, BPC], F32)

        for b in range(BPC):
            # ---- Pass 1(b): cast-load all tiles, reduce as they land ----
            cached = []
            run = runp.tile([P, 4, C], F16, tag="run")
            psum_s = psum2.tile([1, 8 * C], F32, tag="psum_s")
            for t in range(T):
                tl = cache.tile([P, KF, C], F16, tag=f"c{b}_{t}")
                nc.gpsimd.dma_start(out=tl[:], in_=feat_t[b, t])
                cached.append(tl)
                # Later batches' reduce work is pushed back in the scheduler's
                # model clock so the previous batch's multiplies are ordered
                # AHEAD of it on DVE (the model thinks loads are much faster
                # than they are and would otherwise front-load these,
                # head-of-line-blocking the store stream's feed).
                with tc.tile_wait_until(b) if b else nullcontext():
                    # per-tile max tree 32 -> 16 -> 8 -> 4 rows (2x TT mode),
                    # folded into the batch's running max
                    tr = treep.tile([P, 16, C], F16, tag="tree")
                    nc.vector.tensor_tensor(tr[:], tl[:, :16], tl[:, 16:], MAX)
                    nc.vector.tensor_tensor(tr[:, :8], tr[:, :8], tr[:, 8:], MAX)
                    nc.vector.tensor_tensor(
                        tr[:, :4], tr[:, :4], tr[:, 4:8], MAX
                    )
                    if t == 0:
                        nc.vector.tensor_copy(run[:], tr[:, :4])
                    else:
                        nc.vector.tensor_tensor(run[:], run[:], tr[:, :4], MAX)
                    # sum: PE ones-matmuls, PSUM-accumulated over the batch;
                    # the (row, channel) mix is folded at the end.
                    sv = tl[:].rearrange("p (s r) c -> p s (r c)", s=SEG)
                    for seg in range(SEG):
                        nc.tensor.matmul(
                            psum_s[:],
                            ones_col[:],
                            sv[:, seg],
                            start=(t == 0 and seg == 0),
                            stop=(t == T - 1 and seg == SEG - 1),
                        )
            if b == 0:
                # after the b0 load triggers are queued so it doesn't delay
                # them (make_identity runs on the gpsimd engine)
                masks.make_identity(nc, identity[:])

            # short final tree 4 -> 2 -> 1, then cross-partition via PE
            # transpose + DVE reduce
            with tc.tile_wait_until(b) if b else nullcontext():
                s2t = small.tile([P, 2, C], F16, tag="s2")
                nc.vector.tensor_tensor(s2t[:], run[:, :2], run[:, 2:], MAX)
                maxr = small.tile([P, C], F32, tag="maxr")
                nc.vector.tensor_tensor(maxr[:], s2t[:, 0], s2t[:, 1], MAX)
                mt = psum.tile([C, P], F32, tag="mt")
                nc.tensor.transpose(mt[:], maxr[:], identity[:])
                nc.vector.reduce_max(
                    out=xvec[:, 0, b : b + 1], in_=mt[:], axis=mybir.AxisListType.X
                )
                # fold (row, channel) mix: (1, C, 8) reduce -> (1, C)
                srow = small.tile([1, C], F32, tag="srow")
                nc.vector.reduce_sum(
                    out=srow[:],
                    in_=psum_s[:].rearrange("p (s c) -> p c s", c=C),
                    axis=mybir.AxisListType.X,
                )
                # transpose row->column via K=1 matmul, folding the 1/HW scale
                av = psum.tile([C, 1], F32, tag="av")
                nc.tensor.matmul(av[:], srow[:], one_hw[:], start=True, stop=True)
                nc.vector.tensor_copy(xvec[:, 1, b : b + 1], av[:])

                # ---- Recurrence(b): 7 residual PReLU blocks on (C, 2) ----
                xf = xvec[:, :, b]  # (C, 2): cols = (max, avg)
                for k in range(CONV_NUM):
                    y = psum.tile([C, 2], F32, tag="y")
                    nc.tensor.matmul(y[:], w_sb[:, k, :], xf, start=True, stop=True)
                    pos = small.tile([C, 2], F32, tag="pos")
                    nc.vector.tensor_scalar(
                        pos[:], y[:], b_sb[:, k : k + 1], 0.0,
                        mybir.AluOpType.add, mybir.AluOpType.max,
                    )
                    zmin = small.tile([C, 2], F32, tag="zmin")
                    nc.vector.tensor_scalar(
                        zmin[:], y[:], b_sb[:, k : k + 1], 0.0,
                        mybir.AluOpType.add, mybir.AluOpType.min,
                    )
                    pn = small.tile([C, 2], F32, tag="pn")
                    nc.vector.scalar_tensor_tensor(
                        pn[:], zmin[:], a_sb[:, k : k + 1], pos[:],
                        mybir.AluOpType.mult, mybir.AluOpType.add,
                    )
                    xn = small.tile([C, 2], F32, tag="xn")
                    nc.vector.tensor_add(xn[:], pn[:], xf)
                    xf = xn[:]

                # scores(b) = sigmoid(x_max + x_avg): (C, 1)
                ssum = small.tile([C, 1], F32, tag="ssum")
                nc.vector.tensor_add(ssum[:], xf[:, 0:1], xf[:, 1:2])
                scores = small.tile([C, 1], F32, tag="scores")
                nc.scalar.activation(
                    out=scores[:], in_=ssum[:],
                    func=mybir.ActivationFunctionType.Sigmoid,
                )
                # broadcast to all partitions on-chip: (C,1) -T-> (1,C), then
                # a K=1 ones matmul fans it out to (P, C); widen to a full
                # fp16 (P, KF, C) tile so the pass-2 muls see dense step-1
                # operands.
                sc_t = psum.tile([1, C], F32, tag="sc_t")
                nc.tensor.transpose(sc_t[:], scores[:], identity[:C, :C])
                sc_sb = small.tile([1, C], F32, tag="sc_sb")
                nc.vector.tensor_copy(sc_sb[:], sc_t[:])
                bc_ps = psum.tile([P, C], F32, tag="bc")
                nc.tensor.matmul(
                    bc_ps[:], ones_row[:], sc_sb[:], start=True, stop=True
                )
                bcb = small.tile([P, C], F16, tag="bcb")
                nc.vector.tensor_copy(bcb[:], bc_ps[:])
                sct = sctp.tile([P, KF, C], F16, tag="sct")
                nc.vector.tensor_copy(
                    sct[:], bcb[:].unsqueeze(1).to_broadcast([P, KF, C])
                )

                # ---- Pass 2(b): 16-bit multiply in place, ACT widens to fp32
                # staging, stores ride the HWDGE (SP) ring so the write stream
                # overlaps the next batch's SWDGE read stream.
                for t in range(T):
                    nc.vector.tensor_mul(cached[t][:], cached[t][:], sct[:])
                    stg = stgp.tile([P, KF, C], F32, tag="stg")
                    nc.scalar.copy(out=stg[:], in_=cached[t][:])
                    nc.sync.dma_start(out=out_t[b, t], in_=stg[:])

    _split_dma_waits(nc)
    return nc


def _prep_inputs(features, W1, b1, a1):
    feats = np.ascontiguousarray(features, dtype=np.float32).reshape(B, HW, C)
    # lhsT layout: wT[c_in, k, c_out] = W1[k, c_out, c_in]
    wT = np.ascontiguousarray(np.transpose(np.asarray(W1, np.float32), (2, 0, 1)))
    bT = np.ascontiguousarray(np.asarray(b1, np.float32).T)            # (C, 7)
    aT = np.ascontiguousarray(
        np.broadcast_to(np.asarray(a1, np.float32), (C, CONV_NUM))
    )
    return feats, wT, bT, aT


def kernel(features, W1, b1, a1):
    global LAST_EXEC_NS
    feats, wT, bT, aT = _prep_inputs(features, W1, b1, a1)
    nc = _build_nc()
    in_maps = [
        {
            "features": feats[i * BPC : (i + 1) * BPC],
            "wT": wT,
            "bT": bT,
            "aT": aT,
        }
        for i in range(NCORES)
    ]
    import os

    res = run_bass_kernel_spmd(
        nc,
        in_maps,
        list(range(NCORES)),
        trace=PROFILE,
        tmpdir=os.environ.get("BASS_TMPDIR"),
    )
    global LAST_RESULTS
    LAST_RESULTS = res
    LAST_EXEC_NS = res.exec_time_ns
    out = np.concatenate(
        [res.results[i]["out"].reshape(BPC, H, W, C) for i in range(NCORES)], axis=0
    )
    return out


# revision 12
# speedup vs baseline: 1.0324x; 1.0157x over previous
"""Trainium2 Bass kernel for ChannelFeatures (channel-attention style module).

Computes, per batch element b:
    x_max[b] = max over (H,W) of features[b]          # (C,)
    x_avg[b] = mean over (H,W) of features[b]         # (C,)
    7 residual blocks (shared weights on both branches):
        x = prelu(W1[k] @ x + b1[k], a1[k]) + x
    scores[b] = sigmoid(x_max[b] + x_avg[b])          # (C,)
    out[b] = features[b] * scores[b]                  # broadcast over (H,W)

Sharding: pure data parallel over batch — 16 batch elements across 8 cores,
2 per core, weights replicated. No cross-core communication.

Device strategy per core (2 batch elements, each (65536, 64) fp32):
  The kernel is HBM-bound: 33.5 MB in + 33.5 MB out per core, and the two
  HBM directions each sustain ~425 GB/s with measurable independence, so the
  structure keeps the read stream (SWDGE ring) and write stream (HWDGE SP
  ring) concurrently busy:

  * Loads: 16x 2 MB SWDGE cast-DMAs (fp32 DRAM -> fp16 SBUF). 2 MB with one
    contiguous 16 KB chunk per partition keeps each DMA at 128 descriptors,
    so the Q7 descriptor-emission stage (~3 us/DMA, the SWDGE completion
    bottleneck) hides under the 4.7 us drain; the whole 32 MB working set
    stays resident as 16 MB fp16.
  * Max: per-tile tensor_tensor max trees (16-bit 2x DVE mode) folded into a
    running (P, 4, C) max — the per-batch serial tail is well under 1 us.
  * Sum: PE ones-matmuls off the fp16 tiles, PSUM-accumulated per batch
    (mean scale folded into the row->column transpose matmul).
  * Recurrence: 1 PE matmul + 4 DVE ops per block (bias folded into
    tensor_scalar, a*neg+pos fused via scalar_tensor_tensor).
  * Scores broadcast to 128 partitions via a K=1 ones matmul on PE.
  * Pass 2: in-place fp16 multiply (2x mode) -> ACT widens into a 5-deep
    fp32 staging pool -> HWDGE stores. Batch 0's multiplies are ordered
    AHEAD of batch 1's tree ops on DVE (tile_wait_until pushes the latter
    back in the scheduler's model clock, which otherwise trusts its
    2.4x-too-fast DMA model), so the write stream starts as soon as batch
    0's scores exist and overlaps batch 1's read stream.
"""

import numpy as np
from contextlib import ExitStack, nullcontext

import concourse.bass as bass
import concourse.tile as tile
from concourse import masks, mybir
from concourse.bass_utils import run_bass_kernel_spmd

# Problem shapes (hardcoded per contract)
B, H, W, C = 16, 256, 256, 64
CONV_NUM = 7
NCORES = 8
BPC = B // NCORES          # batch elements per core
HW = H * W                 # 65536 spatial positions
P = 128                    # SBUF partitions
KF = 64                    # spatial rows per partition per tile (2 MB tiles:
                           # one contiguous 16 KB chunk per partition keeps the
                           # SWDGE descriptor count at 128/DMA, so descriptor
                           # emission hides under the 4.7 us drain)
KO = 32                    # rows per 1 MB store chunk
TILE_ROWS = P * KF         # 8192 spatial rows per tile
T = HW // TILE_ROWS        # 8 tiles per batch element
F32 = mybir.dt.float32
F16 = mybir.dt.float16     # fp16: 16-bit DVE/PE fast paths, 4x bf16 mantissa

# test.py hooks: set PROFILE=True before calling kernel() to capture an NTFF
# trace; LAST_EXEC_NS then holds the max per-core HW execution time.
PROFILE = False
LAST_EXEC_NS = None
LAST_RESULTS = None


def _split_dma_waits(nc: bass.Bass) -> None:
    """The pinned walrus build rejects DMA instructions carrying more than one
    sync-wait ("Too many sync wait commands"). Tile's sem assignment is not
    transitively minimal, so slot-reuse instructions can get two waits
    (consumer release + WAW with the previous writer). Hoist all but the last
    wait onto wait-only EventSemaphore instructions on the same engine right
    before the instruction."""
    n = 0
    # num=200: outside every id Tile allocated (its end-of-kernel range-clear
    # covers the allocated block), so no collision with released Tile sems.
    dummy = nc.alloc_semaphore(name="wsplit_dummy", num=200)
    for fn in nc.m.functions:
        for blk in fn.blocks:
            new_insts = []
            for inst in blk.instructions:
                si = getattr(inst, "sync_info", None)
                if si is not None and len(si.on_wait) > 1:
                    for w in si.on_wait[:-1]:
                        ev = mybir.InstEventSemaphore(
                            name=f"WSPLIT-{n}", ins=[], outs=[]
                        )
                        n += 1
                        ev.engine = inst.engine
                        # Tick a dedicated dummy sem nobody waits on, so the
                        # simulator/race tooling (which require every
                        # instruction to carry an update) accept the carrier.
                        upd = mybir.SyncUpdate(
                            sync_type="semaphore",
                            id=dummy.num,
                            ant_name=dummy.name,
                            update_mode="sem-add-imm",
                            update_value=1,
                        )
                        ev.sync_info = mybir.SyncInfo(on_wait=[w], on_update=[upd])
                        new_insts.append(ev)
                    si.on_wait = [si.on_wait[-1]]
                new_insts.append(inst)
            blk.instructions = new_insts


def _build_nc() -> bass.Bass:
    nc = bass.Bass()
    feat = nc.declare_dram_parameter("features", [BPC, HW, C], F32, isOutput=False)
    wT = nc.declare_dram_parameter("wT", [C, CONV_NUM, C], F32, isOutput=False)
    bT = nc.declare_dram_parameter("bT", [C, CONV_NUM], F32, isOutput=False)
    aT = nc.declare_dram_parameter("aT", [C, CONV_NUM], F32, isOutput=False)
    out = nc.declare_dram_parameter("out", [BPC, HW, C], F32, isOutput=True)

    feat_t = feat[:].rearrange("b (t p k) c -> b t p k c", p=P, k=KF)
    out_t = out[:].rearrange("b (t p k) c -> b t p k c", p=P, k=KF)

    SEG = KF // 8            # 512-wide matmul segments per tile
    MAX = mybir.AluOpType.max

    with ExitStack() as ctx:
        tc = ctx.enter_context(tile.TileContext(nc))
        singles = ctx.enter_context(tc.tile_pool(name="singles", bufs=1))
        cache = ctx.enter_context(tc.tile_pool(name="cache", bufs=1))
        stgp = ctx.enter_context(tc.tile_pool(name="stgp", bufs=4))
        treep = ctx.enter_context(tc.tile_pool(name="treep", bufs=1))
        runp = ctx.enter_context(tc.tile_pool(name="runp", bufs=2))
        sctp = ctx.enter_context(tc.tile_pool(name="sctp", bufs=2))
        small = ctx.enter_context(tc.tile_pool(name="small", bufs=2))
        psum = ctx.enter_context(tc.tile_pool(name="psum", bufs=1, space="PSUM"))
        psum2 = ctx.enter_context(tc.tile_pool(name="psum2", bufs=2, space="PSUM"))

        # Constants (HWDGE loads; the SWDGE/POOL queue stays clear for tiles)
        w_sb = singles.tile([C, CONV_NUM, C], F32)   # [c_in, k, c_out]
        nc.sync.dma_start(out=w_sb[:], in_=wT[:])
        b_sb = singles.tile([C, CONV_NUM], F32)      # [c, k]
        nc.sync.dma_start(out=b_sb[:], in_=bT[:])
        a_sb = singles.tile([C, CONV_NUM], F32)      # [c, k] (a1[k] per row)
        nc.sync.dma_start(out=a_sb[:], in_=aT[:])
        ones_col = singles.tile([P, 1], F16)
        nc.vector.memset(ones_col[:], 1.0)
        ones_row = singles.tile([1, P], F32)
        nc.vector.memset(ones_row[:], 1.0)
        one_hw = singles.tile([1, 1], F32)
        nc.vector.memset(one_hw[:], 1.0 / HW)
        identity = singles.tile([P, P], F32)

        # [channel, branch(0=max,1=avg), batch]
        xvec = singles.tile([C, 2, BPC], F32)

        for b in range(BPC):
            # ---- Pass 1(b): cast-load all tiles, reduce as they land ----
            cached = []
            run = runp.tile([P, 4, C], F16, tag="run")
            psum_s = psum2.tile([1, 8 * C], F32, tag="psum_s")
            for t in range(T):
                tl = cache.tile([P, KF, C], F16, tag=f"c{b}_{t}")
                nc.gpsimd.dma_start(out=tl[:], in_=feat_t[b, t])
                cached.append(tl)
                # Later batches' reduce work is pushed back in the scheduler's
                # model clock so the previous batch's multiplies are ordered
                # AHEAD of it on DVE (the model thinks loads are much faster
                # than they are and would otherwise front-load these,
                # head-of-line-blocking the store stream's feed).
                with tc.tile_wait_until(b) if b else nullcontext():
                    # per-tile max tree 64 -> 32 -> 16 -> 8 -> 4 rows (2x
                    # TT mode), folded into the batch's running max
                    tr = treep.tile([P, 32, C], F16, tag="tree")
                    nc.vector.tensor_tensor(tr[:], tl[:, :32], tl[:, 32:], MAX)
                    nc.vector.tensor_tensor(
                        tr[:, :16], tr[:, :16], tr[:, 16:], MAX
                    )
                    nc.vector.tensor_tensor(
                        tr[:, :8], tr[:, :8], tr[:, 8:16], MAX
                    )
                    nc.vector.tensor_tensor(
                        tr[:, :4], tr[:, :4], tr[:, 4:8], MAX
                    )
                    if t == 0:
                        nc.vector.tensor_copy(run[:], tr[:, :4])
                    else:
                        nc.vector.tensor_tensor(run[:], run[:], tr[:, :4], MAX)
                    # sum: PE ones-matmuls, PSUM-accumulated over the batch;
                    # the (row, channel) mix is folded at the end.
                    sv = tl[:].rearrange("p (s r) c -> p s (r c)", s=SEG)
                    for seg in range(SEG):
                        nc.tensor.matmul(
                            psum_s[:],
                            ones_col[:],
                            sv[:, seg],
                            start=(t == 0 and seg == 0),
                            stop=(t == T - 1 and seg == SEG - 1),
                        )
            if b == 0:
                # after the b0 load triggers are queued so it doesn't delay
                # them (make_identity runs on the gpsimd engine)
                masks.make_identity(nc, identity[:])

            # short final tree 4 -> 2 -> 1, then cross-partition via PE
            # transpose + DVE reduce
            with tc.tile_wait_until(b) if b else nullcontext():
                s2t = small.tile([P, 2, C], F16, tag="s2")
                nc.vector.tensor_tensor(s2t[:], run[:, :2], run[:, 2:], MAX)
                maxr = small.tile([P, C], F32, tag="maxr")
                nc.vector.tensor_tensor(maxr[:], s2t[:, 0], s2t[:, 1], MAX)
                mt = psum.tile([C, P], F32, tag="mt")
                nc.tensor.transpose(mt[:], maxr[:], identity[:])
                nc.vector.reduce_max(
                    out=xvec[:, 0, b : b + 1], in_=mt[:], axis=mybir.AxisListType.X
                )
                # fold (row, channel) mix: (1, C, 8) reduce -> (1, C)
                srow = small.tile([1, C], F32, tag="srow")
                nc.vector.reduce_sum(
                    out=srow[:],
                    in_=psum_s[:].rearrange("p (s c) -> p c s", c=C),
                    axis=mybir.AxisListType.X,
                )
                # transpose row->column via K=1 matmul, folding the 1/HW scale
                av = psum.tile([C, 1], F32, tag="av")
                nc.tensor.matmul(av[:], srow[:], one_hw[:], start=True, stop=True)
                nc.vector.tensor_copy(xvec[:, 1, b : b + 1], av[:])

                # ---- Recurrence(b): 7 residual PReLU blocks on (C, 2) ----
                xf = xvec[:, :, b]  # (C, 2): cols = (max, avg)
                for k in range(CONV_NUM):
                    y = psum.tile([C, 2], F32, tag="y")
                    nc.tensor.matmul(y[:], w_sb[:, k, :], xf, start=True, stop=True)
                    pos = small.tile([C, 2], F32, tag="pos")
                    nc.vector.tensor_scalar(
                        pos[:], y[:], b_sb[:, k : k + 1], 0.0,
                        mybir.AluOpType.add, mybir.AluOpType.max,
                    )
                    zmin = small.tile([C, 2], F32, tag="zmin")
                    nc.vector.tensor_scalar(
                        zmin[:], y[:], b_sb[:, k : k + 1], 0.0,
                        mybir.AluOpType.add, mybir.AluOpType.min,
                    )
                    pn = small.tile([C, 2], F32, tag="pn")
                    nc.vector.scalar_tensor_tensor(
                        pn[:], zmin[:], a_sb[:, k : k + 1], pos[:],
                        mybir.AluOpType.mult, mybir.AluOpType.add,
                    )
                    xn = small.tile([C, 2], F32, tag="xn")
                    nc.vector.tensor_add(xn[:], pn[:], xf)
                    xf = xn[:]

                # scores(b) = sigmoid(x_max + x_avg): (C, 1)
                ssum = small.tile([C, 1], F32, tag="ssum")
                nc.vector.tensor_add(ssum[:], xf[:, 0:1], xf[:, 1:2])
                scores = small.tile([C, 1], F32, tag="scores")
                nc.scalar.activation(
                    out=scores[:], in_=ssum[:],
                    func=mybir.ActivationFunctionType.Sigmoid,
                )
                # broadcast to all partitions on-chip: (C,1) -T-> (1,C), then
                # a K=1 ones matmul fans it out to (P, C); widen to a full
                # fp16 (P, KF, C) tile so the pass-2 muls see dense step-1
                # operands.
                sc_t = psum.tile([1, C], F32, tag="sc_t")
                nc.tensor.transpose(sc_t[:], scores[:], identity[:C, :C])
                sc_sb = small.tile([1, C], F32, tag="sc_sb")
                nc.vector.tensor_copy(sc_sb[:], sc_t[:])
                bc_ps = psum.tile([P, C], F32, tag="bc")
                nc.tensor.matmul(
                    bc_ps[:], ones_row[:], sc_sb[:], start=True, stop=True
                )
                bcb = small.tile([P, C], F16, tag="bcb")
                nc.vector.tensor_copy(bcb[:], bc_ps[:])
                sct = sctp.tile([P, KF, C], F16, tag="sct")
                nc.vector.tensor_copy(
                    sct[:], bcb[:].unsqueeze(1).to_broadcast([P, KF, C])
                )

                # ---- Pass 2(b): 16-bit multiply in place, ACT widens to fp32
                # staging, stores ride the HWDGE (SP) ring so the write stream
                # overlaps the next batch's SWDGE read stream.
                for t in range(T):
                    nc.vector.tensor_mul(cached[t][:], cached[t][:], sct[:])
                    for h in range(KF // KO):
                        stg = stgp.tile([P, KO, C], F32, tag="stg")
                        nc.scalar.copy(
                            out=stg[:],
                            in_=cached[t][:, h * KO : (h + 1) * KO],
                        )
                        nc.sync.dma_start(
                            out=out_t[b, t, :, h * KO : (h + 1) * KO],
                            in_=stg[:],
                        )

    _split_dma_waits(nc)
    return nc


def _prep_inputs(features, W1, b1, a1):
    feats = np.ascontiguousarray(features, dtype=np.float32).reshape(B, HW, C)
    # lhsT layout: wT[c_in, k, c_out] = W1[k, c_out, c_in]
    wT = np.ascontiguousarray(np.transpose(np.asarray(W1, np.float32), (2, 0, 1)))
    bT = np.ascontiguousarray(np.asarray(b1, np.float32).T)            # (C, 7)
    aT = np.ascontiguousarray(
        np.broadcast_to(np.asarray(a1, np.float32), (C, CONV_NUM))
    )
    return feats, wT, bT, aT


def kernel(features, W1, b1, a1):
    global LAST_EXEC_NS
    feats, wT, bT, aT = _prep_inputs(features, W1, b1, a1)
    nc = _build_nc()
    in_maps = [
        {
            "features": feats[i * BPC : (i + 1) * BPC],
            "wT": wT,
            "bT": bT,
            "aT": aT,
        }
        for i in range(NCORES)
    ]
    import os

    res = run_bass_kernel_spmd(
        nc,
        in_maps,
        list(range(NCORES)),
        trace=PROFILE,
        tmpdir=os.environ.get("BASS_TMPDIR"),
    )
    global LAST_RESULTS
    LAST_RESULTS = res
    LAST_EXEC_NS = res.exec_time_ns
    out = np.concatenate(
        [res.results[i]["out"].reshape(BPC, H, W, C) for i in range(NCORES)], axis=0
    )
    return out


# revision 13
# speedup vs baseline: 1.0380x; 1.0054x over previous
"""Trainium2 Bass kernel for ChannelFeatures (channel-attention style module).

Computes, per batch element b:
    x_max[b] = max over (H,W) of features[b]          # (C,)
    x_avg[b] = mean over (H,W) of features[b]         # (C,)
    7 residual blocks (shared weights on both branches):
        x = prelu(W1[k] @ x + b1[k], a1[k]) + x
    scores[b] = sigmoid(x_max[b] + x_avg[b])          # (C,)
    out[b] = features[b] * scores[b]                  # broadcast over (H,W)

Sharding: pure data parallel over batch — 16 batch elements across 8 cores,
2 per core, weights replicated. No cross-core communication.

Device strategy per core (2 batch elements, each (65536, 64) fp32):
  The kernel is HBM-bound: 33.5 MB in + 33.5 MB out per core. All bulk data
  rides the SWDGE ring as 1 MB cast-DMAs at ~420 GB/s (1 MB is the SWDGE
  sweet spot; bigger chunked transfers hit the Q7 descriptor-emission
  ceiling), and every HBM byte moves exactly once:

  * Loads: 32x 1 MB SWDGE cast-DMAs (fp32 DRAM -> fp16 SBUF); the whole
    32 MB working set stays resident as 16 MB fp16.
  * Max: per-tile tensor_tensor max trees (16-bit 2x DVE mode) folded into a
    running (P, 4, C) max — the per-batch serial tail is well under 1 us.
  * Sum: PE ones-matmuls off the fp16 tiles, PSUM-accumulated per batch
    (mean scale folded into the row->column transpose matmul).
  * Recurrence: 1 PE matmul + 4 DVE ops per block (bias folded into
    tensor_scalar, a*neg+pos fused via scalar_tensor_tensor).
  * Scores broadcast to 128 partitions via a K=1 ones matmul on PE.
  * Pass 2: fp16 multiply (2x mode) into fp16 staging, then SWDGE
    cast-stores (fp16 SBUF -> fp32 DRAM) that queue up behind the loads on
    the ring, so the write phase starts the moment the read phase drains.
    Batch 0's multiplies are ordered AHEAD of batch 1's tree ops on DVE
    (tile_wait_until pushes the latter back in the scheduler's model clock)
    so the store triggers are enqueued well before the ring needs them;
    batch 1's tree ops lose nothing since its load completions are gated by
    the SWDGE descriptor-emission cadence anyway.
"""

import numpy as np
from contextlib import ExitStack, nullcontext

import concourse.bass as bass
import concourse.tile as tile
from concourse import masks, mybir
from concourse.bass_utils import run_bass_kernel_spmd

# Problem shapes (hardcoded per contract)
B, H, W, C = 16, 256, 256, 64
CONV_NUM = 7
NCORES = 8
BPC = B // NCORES          # batch elements per core
HW = H * W                 # 65536 spatial positions
P = 128                    # SBUF partitions
KF = 32                    # spatial rows per partition per tile (1 MB is the
                           # SWDGE sweet spot: 2 MB cast-DMAs run ~25% slower)
TILE_ROWS = P * KF         # 4096 spatial rows per tile
T = HW // TILE_ROWS        # 16 tiles per batch element
F32 = mybir.dt.float32
F16 = mybir.dt.float16     # fp16: 16-bit DVE/PE fast paths, 4x bf16 mantissa

# test.py hooks: set PROFILE=True before calling kernel() to capture an NTFF
# trace; LAST_EXEC_NS then holds the max per-core HW execution time.
PROFILE = False
LAST_EXEC_NS = None
LAST_RESULTS = None


def _split_dma_waits(nc: bass.Bass) -> None:
    """The pinned walrus build rejects DMA instructions carrying more than one
    sync-wait ("Too many sync wait commands"). Tile's sem assignment is not
    transitively minimal, so slot-reuse instructions can get two waits
    (consumer release + WAW with the previous writer). Hoist all but the last
    wait onto wait-only EventSemaphore instructions on the same engine right
    before the instruction."""
    n = 0
    # num=200: outside every id Tile allocated (its end-of-kernel range-clear
    # covers the allocated block), so no collision with released Tile sems.
    dummy = nc.alloc_semaphore(name="wsplit_dummy", num=200)
    for fn in nc.m.functions:
        for blk in fn.blocks:
            new_insts = []
            for inst in blk.instructions:
                si = getattr(inst, "sync_info", None)
                if si is not None and len(si.on_wait) > 1:
                    for w in si.on_wait[:-1]:
                        ev = mybir.InstEventSemaphore(
                            name=f"WSPLIT-{n}", ins=[], outs=[]
                        )
                        n += 1
                        ev.engine = inst.engine
                        # Tick a dedicated dummy sem nobody waits on, so the
                        # simulator/race tooling (which require every
                        # instruction to carry an update) accept the carrier.
                        upd = mybir.SyncUpdate(
                            sync_type="semaphore",
                            id=dummy.num,
                            ant_name=dummy.name,
                            update_mode="sem-add-imm",
                            update_value=1,
                        )
                        ev.sync_info = mybir.SyncInfo(on_wait=[w], on_update=[upd])
                        new_insts.append(ev)
                    si.on_wait = [si.on_wait[-1]]
                new_insts.append(inst)
            blk.instructions = new_insts


def _build_nc() -> bass.Bass:
    nc = bass.Bass()
    feat = nc.declare_dram_parameter("features", [BPC, HW, C], F32, isOutput=False)
    wT = nc.declare_dram_parameter("wT", [C, CONV_NUM, C], F32, isOutput=False)
    bT = nc.declare_dram_parameter("bT", [C, CONV_NUM], F32, isOutput=False)
    aT = nc.declare_dram_parameter("aT", [C, CONV_NUM], F32, isOutput=False)
    out = nc.declare_dram_parameter("out", [BPC, HW, C], F32, isOutput=True)

    feat_t = feat[:].rearrange("b (t p k) c -> b t p k c", p=P, k=KF)
    out_t = out[:].rearrange("b (t p k) c -> b t p k c", p=P, k=KF)

    SEG = KF // 8            # 512-wide matmul segments per tile
    MAX = mybir.AluOpType.max

    with ExitStack() as ctx:
        tc = ctx.enter_context(tile.TileContext(nc))
        singles = ctx.enter_context(tc.tile_pool(name="singles", bufs=1))
        cache = ctx.enter_context(tc.tile_pool(name="cache", bufs=1))
        stgp = ctx.enter_context(tc.tile_pool(name="stgp", bufs=4))
        treep = ctx.enter_context(tc.tile_pool(name="treep", bufs=2))
        runp = ctx.enter_context(tc.tile_pool(name="runp", bufs=2))
        sctp = ctx.enter_context(tc.tile_pool(name="sctp", bufs=2))
        small = ctx.enter_context(tc.tile_pool(name="small", bufs=2))
        psum = ctx.enter_context(tc.tile_pool(name="psum", bufs=1, space="PSUM"))
        psum2 = ctx.enter_context(tc.tile_pool(name="psum2", bufs=2, space="PSUM"))

        # Constants (HWDGE loads; the SWDGE/POOL queue stays clear for tiles)
        w_sb = singles.tile([C, CONV_NUM, C], F32)   # [c_in, k, c_out]
        nc.sync.dma_start(out=w_sb[:], in_=wT[:])
        b_sb = singles.tile([C, CONV_NUM], F32)      # [c, k]
        nc.sync.dma_start(out=b_sb[:], in_=bT[:])
        a_sb = singles.tile([C, CONV_NUM], F32)      # [c, k] (a1[k] per row)
        nc.sync.dma_start(out=a_sb[:], in_=aT[:])
        ones_col = singles.tile([P, 1], F16)
        nc.vector.memset(ones_col[:], 1.0)
        ones_row = singles.tile([1, P], F32)
        nc.vector.memset(ones_row[:], 1.0)
        one_hw = singles.tile([1, 1], F32)
        nc.vector.memset(one_hw[:], 1.0 / HW)
        identity = singles.tile([P, P], F32)

        # [channel, branch(0=max,1=avg), batch]
        xvec = singles.tile([C, 2, BPC], F32)

        for b in range(BPC):
            # ---- Pass 1(b): cast-load all tiles, reduce as they land ----
            cached = []
            run = runp.tile([P, 4, C], F16, tag="run")
            psum_s = psum2.tile([1, 8 * C], F32, tag="psum_s")
            for t in range(T):
                tl = cache.tile([P, KF, C], F16, tag=f"c{b}_{t}")
                nc.gpsimd.dma_start(out=tl[:], in_=feat_t[b, t])
                cached.append(tl)
                # Later batches' reduce work is pushed back in the scheduler's
                # model clock so the previous batch's multiplies are ordered
                # AHEAD of it on DVE (the model thinks loads are much faster
                # than they are and would otherwise front-load these,
                # head-of-line-blocking the store stream's feed).
                with tc.tile_wait_until(b) if b else nullcontext():
                    # per-tile max tree 32 -> 16 -> 8 -> 4 rows (2x TT
                    # mode), folded into the batch's running max
                    tr = treep.tile([P, 16, C], F16, tag="tree")
                    nc.vector.tensor_tensor(tr[:], tl[:, :16], tl[:, 16:], MAX)
                    nc.vector.tensor_tensor(tr[:, :8], tr[:, :8], tr[:, 8:], MAX)
                    nc.vector.tensor_tensor(
                        tr[:, :4], tr[:, :4], tr[:, 4:8], MAX
                    )
                    if t == 0:
                        nc.vector.tensor_copy(run[:], tr[:, :4])
                    else:
                        nc.vector.tensor_tensor(run[:], run[:], tr[:, :4], MAX)
                    # sum: PE ones-matmuls, PSUM-accumulated over the batch;
                    # the (row, channel) mix is folded at the end.
                    sv = tl[:].rearrange("p (s r) c -> p s (r c)", s=SEG)
                    for seg in range(SEG):
                        nc.tensor.matmul(
                            psum_s[:],
                            ones_col[:],
                            sv[:, seg],
                            start=(t == 0 and seg == 0),
                            stop=(t == T - 1 and seg == SEG - 1),
                        )
            if b == 0:
                # after the b0 load triggers are queued so it doesn't delay
                # them (make_identity runs on the gpsimd engine)
                masks.make_identity(nc, identity[:])

            # short final tree 4 -> 2 -> 1, then cross-partition via PE
            # transpose + DVE reduce
            with tc.tile_wait_until(b) if b else nullcontext():
                s2t = small.tile([P, 2, C], F16, tag="s2")
                nc.vector.tensor_tensor(s2t[:], run[:, :2], run[:, 2:], MAX)
                maxr = small.tile([P, C], F32, tag="maxr")
                nc.vector.tensor_tensor(maxr[:], s2t[:, 0], s2t[:, 1], MAX)
                mt = psum.tile([C, P], F32, tag="mt")
                nc.tensor.transpose(mt[:], maxr[:], identity[:])
                nc.vector.reduce_max(
                    out=xvec[:, 0, b : b + 1], in_=mt[:], axis=mybir.AxisListType.X
                )
                # fold (row, channel) mix: (1, C, 8) reduce -> (1, C)
                srow = small.tile([1, C], F32, tag="srow")
                nc.vector.reduce_sum(
                    out=srow[:],
                    in_=psum_s[:].rearrange("p (s c) -> p c s", c=C),
                    axis=mybir.AxisListType.X,
                )
                # transpose row->column via K=1 matmul, folding the 1/HW scale
                av = psum.tile([C, 1], F32, tag="av")
                nc.tensor.matmul(av[:], srow[:], one_hw[:], start=True, stop=True)
                nc.vector.tensor_copy(xvec[:, 1, b : b + 1], av[:])

                # ---- Recurrence(b): 7 residual PReLU blocks on (C, 2) ----
                xf = xvec[:, :, b]  # (C, 2): cols = (max, avg)
                for k in range(CONV_NUM):
                    y = psum.tile([C, 2], F32, tag="y")
                    nc.tensor.matmul(y[:], w_sb[:, k, :], xf, start=True, stop=True)
                    pos = small.tile([C, 2], F32, tag="pos")
                    nc.vector.tensor_scalar(
                        pos[:], y[:], b_sb[:, k : k + 1], 0.0,
                        mybir.AluOpType.add, mybir.AluOpType.max,
                    )
                    zmin = small.tile([C, 2], F32, tag="zmin")
                    nc.vector.tensor_scalar(
                        zmin[:], y[:], b_sb[:, k : k + 1], 0.0,
                        mybir.AluOpType.add, mybir.AluOpType.min,
                    )
                    pn = small.tile([C, 2], F32, tag="pn")
                    nc.vector.scalar_tensor_tensor(
                        pn[:], zmin[:], a_sb[:, k : k + 1], pos[:],
                        mybir.AluOpType.mult, mybir.AluOpType.add,
                    )
                    xn = small.tile([C, 2], F32, tag="xn")
                    nc.vector.tensor_add(xn[:], pn[:], xf)
                    xf = xn[:]

                # scores(b) = sigmoid(x_max + x_avg): (C, 1)
                ssum = small.tile([C, 1], F32, tag="ssum")
                nc.vector.tensor_add(ssum[:], xf[:, 0:1], xf[:, 1:2])
                scores = small.tile([C, 1], F32, tag="scores")
                nc.scalar.activation(
                    out=scores[:], in_=ssum[:],
                    func=mybir.ActivationFunctionType.Sigmoid,
                )
                # broadcast to all partitions on-chip: (C,1) -T-> (1,C), then
                # a K=1 ones matmul fans it out to (P, C); widen to a full
                # fp16 (P, KF, C) tile so the pass-2 muls see dense step-1
                # operands.
                sc_t = psum.tile([1, C], F32, tag="sc_t")
                nc.tensor.transpose(sc_t[:], scores[:], identity[:C, :C])
                sc_sb = small.tile([1, C], F32, tag="sc_sb")
                nc.vector.tensor_copy(sc_sb[:], sc_t[:])
                bc_ps = psum.tile([P, C], F32, tag="bc")
                nc.tensor.matmul(
                    bc_ps[:], ones_row[:], sc_sb[:], start=True, stop=True
                )
                bcb = small.tile([P, C], F16, tag="bcb")
                nc.vector.tensor_copy(bcb[:], bc_ps[:])
                sct = sctp.tile([P, KF, C], F16, tag="sct")
                nc.vector.tensor_copy(
                    sct[:], bcb[:].unsqueeze(1).to_broadcast([P, KF, C])
                )

                # ---- Pass 2(b): 16-bit multiply in place, ACT widens to fp32
                # staging, stores ride the HWDGE (SP) ring so the write stream
                # overlaps the next batch's SWDGE read stream.
                for t in range(T):
                    stg = stgp.tile([P, KF, C], F16, tag="stg")
                    nc.vector.tensor_mul(stg[:], cached[t][:], sct[:])
                    nc.gpsimd.dma_start(out=out_t[b, t], in_=stg[:])

    _split_dma_waits(nc)
    return nc


def _prep_inputs(features, W1, b1, a1):
    feats = np.ascontiguousarray(features, dtype=np.float32).reshape(B, HW, C)
    # lhsT layout: wT[c_in, k, c_out] = W1[k, c_out, c_in]
    wT = np.ascontiguousarray(np.transpose(np.asarray(W1, np.float32), (2, 0, 1)))
    bT = np.ascontiguousarray(np.asarray(b1, np.float32).T)            # (C, 7)
    aT = np.ascontiguousarray(
        np.broadcast_to(np.asarray(a1, np.float32), (C, CONV_NUM))
    )
    return feats, wT, bT, aT


def kernel(features, W1, b1, a1):
    global LAST_EXEC_NS
    feats, wT, bT, aT = _prep_inputs(features, W1, b1, a1)
    nc = _build_nc()
    in_maps = [
        {
            "features": feats[i * BPC : (i + 1) * BPC],
            "wT": wT,
            "bT": bT,
            "aT": aT,
        }
        for i in range(NCORES)
    ]
    import os

    res = run_bass_kernel_spmd(
        nc,
        in_maps,
        list(range(NCORES)),
        trace=PROFILE,
        tmpdir=os.environ.get("BASS_TMPDIR"),
    )
    global LAST_RESULTS
    LAST_RESULTS = res
    LAST_EXEC_NS = res.exec_time_ns
    out = np.concatenate(
        [res.results[i]["out"].reshape(BPC, H, W, C) for i in range(NCORES)], axis=0
    )
    return out


# revision 14
# speedup vs baseline: 1.1917x; 1.1481x over previous
"""Trainium2 Bass kernel for ChannelFeatures (channel-attention style module).

Computes, per batch element b:
    x_max[b] = max over (H,W) of features[b]          # (C,)
    x_avg[b] = mean over (H,W) of features[b]         # (C,)
    7 residual blocks (shared weights on both branches):
        x = prelu(W1[k] @ x + b1[k], a1[k]) + x
    scores[b] = sigmoid(x_max[b] + x_avg[b])          # (C,)
    out[b] = features[b] * scores[b]                  # broadcast over (H,W)

Sharding: pure data parallel over batch — 16 batch elements across 8 cores,
2 per core, weights replicated. No cross-core communication.

Device strategy per core (2 batch elements, each (65536, 64) fp32):
  The kernel is HBM-bound: 33.5 MB in + 33.5 MB out per core. All bulk data
  rides the SWDGE ring as 1 MB cast-DMAs at ~420 GB/s (1 MB is the SWDGE
  sweet spot; bigger chunked transfers hit the Q7 descriptor-emission
  ceiling), and every HBM byte moves exactly once:

  * Loads: 32x 1 MB SWDGE cast-DMAs (fp32 DRAM -> fp16 SBUF); the whole
    32 MB working set stays resident as 16 MB fp16.
  * Max: per-tile tensor_tensor max trees (16-bit 2x DVE mode) folded into a
    running (P, 4, C) max — the per-batch serial tail is well under 1 us.
  * Sum: PE ones-matmuls off the fp16 tiles, PSUM-accumulated per batch
    (mean scale folded into the row->column transpose matmul).
  * Recurrence: 1 PE matmul + 4 DVE ops per block (bias folded into
    tensor_scalar, a*neg+pos fused via scalar_tensor_tensor).
  * Scores broadcast to 128 partitions via a K=1 ones matmul on PE.
  * Pass 2: stores ride the HWDGE (SP) ring — RTL descriptor generation,
    no Q7 emission cost — so the write stream overlaps the SWDGE read
    stream at the SDMA engines (measured: the two directions sustain
    ~420 GB/s each with real independence). Batch 0 multiplies in fp16
    (2x mode, DVE is contended then) and widens on the otherwise-idle ACT;
    batch 1 fuses multiply+widen into one fp32-out DVE op (shorter
    pipeline on the critical tail, DVE is free by then). Batch 0's
    multiplies are ordered AHEAD of batch 1's tree ops on DVE
    (tile_wait_until pushes the latter back in the scheduler's model
    clock, which otherwise trusts its 2.4x-too-fast DMA model); batch 1's
    tree ops lose nothing since its load completions are gated by the
    SWDGE descriptor-emission cadence anyway.
"""

import numpy as np
from contextlib import ExitStack, nullcontext

import concourse.bass as bass
import concourse.tile as tile
from concourse import masks, mybir
from concourse.bass_utils import run_bass_kernel_spmd

# Problem shapes (hardcoded per contract)
B, H, W, C = 16, 256, 256, 64
CONV_NUM = 7
NCORES = 8
BPC = B // NCORES          # batch elements per core
HW = H * W                 # 65536 spatial positions
P = 128                    # SBUF partitions
KF = 32                    # spatial rows per partition per tile (1 MB is the
                           # SWDGE sweet spot: 2 MB cast-DMAs run ~25% slower)
TILE_ROWS = P * KF         # 4096 spatial rows per tile
T = HW // TILE_ROWS        # 16 tiles per batch element
F32 = mybir.dt.float32
F16 = mybir.dt.float16     # fp16: 16-bit DVE/PE fast paths, 4x bf16 mantissa

# test.py hooks: set PROFILE=True before calling kernel() to capture an NTFF
# trace; LAST_EXEC_NS then holds the max per-core HW execution time.
PROFILE = False
LAST_EXEC_NS = None
LAST_RESULTS = None


def _split_dma_waits(nc: bass.Bass) -> None:
    """The pinned walrus build rejects DMA instructions carrying more than one
    sync-wait ("Too many sync wait commands"). Tile's sem assignment is not
    transitively minimal, so slot-reuse instructions can get two waits
    (consumer release + WAW with the previous writer). Hoist all but the last
    wait onto wait-only EventSemaphore instructions on the same engine right
    before the instruction."""
    n = 0
    # num=200: outside every id Tile allocated (its end-of-kernel range-clear
    # covers the allocated block), so no collision with released Tile sems.
    dummy = nc.alloc_semaphore(name="wsplit_dummy", num=200)
    for fn in nc.m.functions:
        for blk in fn.blocks:
            new_insts = []
            for inst in blk.instructions:
                si = getattr(inst, "sync_info", None)
                if si is not None and len(si.on_wait) > 1:
                    for w in si.on_wait[:-1]:
                        ev = mybir.InstEventSemaphore(
                            name=f"WSPLIT-{n}", ins=[], outs=[]
                        )
                        n += 1
                        ev.engine = inst.engine
                        # Tick a dedicated dummy sem nobody waits on, so the
                        # simulator/race tooling (which require every
                        # instruction to carry an update) accept the carrier.
                        upd = mybir.SyncUpdate(
                            sync_type="semaphore",
                            id=dummy.num,
                            ant_name=dummy.name,
                            update_mode="sem-add-imm",
                            update_value=1,
                        )
                        ev.sync_info = mybir.SyncInfo(on_wait=[w], on_update=[upd])
                        new_insts.append(ev)
                    si.on_wait = [si.on_wait[-1]]
                new_insts.append(inst)
            blk.instructions = new_insts


def _build_nc() -> bass.Bass:
    nc = bass.Bass()
    feat = nc.declare_dram_parameter("features", [BPC, HW, C], F32, isOutput=False)
    wT = nc.declare_dram_parameter("wT", [C, CONV_NUM, C], F32, isOutput=False)
    bT = nc.declare_dram_parameter("bT", [C, CONV_NUM], F32, isOutput=False)
    aT = nc.declare_dram_parameter("aT", [C, CONV_NUM], F32, isOutput=False)
    out = nc.declare_dram_parameter("out", [BPC, HW, C], F32, isOutput=True)

    feat_t = feat[:].rearrange("b (t p k) c -> b t p k c", p=P, k=KF)
    out_t = out[:].rearrange("b (t p k) c -> b t p k c", p=P, k=KF)

    SEG = KF // 8            # 512-wide matmul segments per tile
    MAX = mybir.AluOpType.max

    with ExitStack() as ctx:
        tc = ctx.enter_context(tile.TileContext(nc))
        singles = ctx.enter_context(tc.tile_pool(name="singles", bufs=1))
        cache = ctx.enter_context(tc.tile_pool(name="cache", bufs=1))
        stgp = ctx.enter_context(tc.tile_pool(name="stgp", bufs=5))
        treep = ctx.enter_context(tc.tile_pool(name="treep", bufs=2))
        runp = ctx.enter_context(tc.tile_pool(name="runp", bufs=2))
        sctp = ctx.enter_context(tc.tile_pool(name="sctp", bufs=2))
        small = ctx.enter_context(tc.tile_pool(name="small", bufs=2))
        psum = ctx.enter_context(tc.tile_pool(name="psum", bufs=1, space="PSUM"))
        psum2 = ctx.enter_context(tc.tile_pool(name="psum2", bufs=2, space="PSUM"))

        # Constants (HWDGE loads; the SWDGE/POOL queue stays clear for tiles)
        w_sb = singles.tile([C, CONV_NUM, C], F32)   # [c_in, k, c_out]
        nc.sync.dma_start(out=w_sb[:], in_=wT[:])
        b_sb = singles.tile([C, CONV_NUM], F32)      # [c, k]
        nc.sync.dma_start(out=b_sb[:], in_=bT[:])
        a_sb = singles.tile([C, CONV_NUM], F32)      # [c, k] (a1[k] per row)
        nc.sync.dma_start(out=a_sb[:], in_=aT[:])
        ones_col = singles.tile([P, 1], F16)
        nc.vector.memset(ones_col[:], 1.0)
        ones_row = singles.tile([1, P], F32)
        nc.vector.memset(ones_row[:], 1.0)
        one_hw = singles.tile([1, 1], F32)
        nc.vector.memset(one_hw[:], 1.0 / HW)
        identity = singles.tile([P, P], F32)

        # [channel, branch(0=max,1=avg), batch]
        xvec = singles.tile([C, 2, BPC], F32)

        for b in range(BPC):
            # ---- Pass 1(b): cast-load all tiles, reduce as they land ----
            cached = []
            run = runp.tile([P, 4, C], F16, tag="run")
            psum_s = psum2.tile([1, 8 * C], F32, tag="psum_s")
            for t in range(T):
                tl = cache.tile([P, KF, C], F16, tag=f"c{b}_{t}")
                nc.gpsimd.dma_start(out=tl[:], in_=feat_t[b, t])
                cached.append(tl)
                # Later batches' reduce work is pushed back in the scheduler's
                # model clock so the previous batch's multiplies are ordered
                # AHEAD of it on DVE (the model thinks loads are much faster
                # than they are and would otherwise front-load these,
                # head-of-line-blocking the store stream's feed).
                with tc.tile_wait_until(b) if b else nullcontext():
                    # per-tile max tree 32 -> 16 -> 8 -> 4 rows (2x TT
                    # mode), folded into the batch's running max
                    tr = treep.tile([P, 16, C], F16, tag="tree")
                    nc.vector.tensor_tensor(tr[:], tl[:, :16], tl[:, 16:], MAX)
                    nc.vector.tensor_tensor(tr[:, :8], tr[:, :8], tr[:, 8:], MAX)
                    nc.vector.tensor_tensor(
                        tr[:, :4], tr[:, :4], tr[:, 4:8], MAX
                    )
                    if t == 0:
                        nc.vector.tensor_copy(run[:], tr[:, :4])
                    else:
                        nc.vector.tensor_tensor(run[:], run[:], tr[:, :4], MAX)
                    # sum: PE ones-matmuls, PSUM-accumulated over the batch;
                    # the (row, channel) mix is folded at the end.
                    sv = tl[:].rearrange("p (s r) c -> p s (r c)", s=SEG)
                    for seg in range(SEG):
                        nc.tensor.matmul(
                            psum_s[:],
                            ones_col[:],
                            sv[:, seg],
                            start=(t == 0 and seg == 0),
                            stop=(t == T - 1 and seg == SEG - 1),
                        )
            if b == 0:
                # after the b0 load triggers are queued so it doesn't delay
                # them (make_identity runs on the gpsimd engine)
                masks.make_identity(nc, identity[:])

            # short final tree 4 -> 2 -> 1, then cross-partition via PE
            # transpose + DVE reduce
            with tc.tile_wait_until(b) if b else nullcontext():
                s2t = small.tile([P, 2, C], F16, tag="s2")
                nc.vector.tensor_tensor(s2t[:], run[:, :2], run[:, 2:], MAX)
                maxr = small.tile([P, C], F32, tag="maxr")
                nc.vector.tensor_tensor(maxr[:], s2t[:, 0], s2t[:, 1], MAX)
                mt = psum.tile([C, P], F32, tag="mt")
                nc.tensor.transpose(mt[:], maxr[:], identity[:])
                nc.vector.reduce_max(
                    out=xvec[:, 0, b : b + 1], in_=mt[:], axis=mybir.AxisListType.X
                )
                # fold (row, channel) mix: (1, C, 8) reduce -> (1, C)
                srow = small.tile([1, C], F32, tag="srow")
                nc.vector.reduce_sum(
                    out=srow[:],
                    in_=psum_s[:].rearrange("p (s c) -> p c s", c=C),
                    axis=mybir.AxisListType.X,
                )
                # transpose row->column via K=1 matmul, folding the 1/HW scale
                av = psum.tile([C, 1], F32, tag="av")
                nc.tensor.matmul(av[:], srow[:], one_hw[:], start=True, stop=True)
                nc.vector.tensor_copy(xvec[:, 1, b : b + 1], av[:])

                # ---- Recurrence(b): 7 residual PReLU blocks on (C, 2) ----
                xf = xvec[:, :, b]  # (C, 2): cols = (max, avg)
                for k in range(CONV_NUM):
                    y = psum.tile([C, 2], F32, tag="y")
                    nc.tensor.matmul(y[:], w_sb[:, k, :], xf, start=True, stop=True)
                    pos = small.tile([C, 2], F32, tag="pos")
                    nc.vector.tensor_scalar(
                        pos[:], y[:], b_sb[:, k : k + 1], 0.0,
                        mybir.AluOpType.add, mybir.AluOpType.max,
                    )
                    zmin = small.tile([C, 2], F32, tag="zmin")
                    nc.vector.tensor_scalar(
                        zmin[:], y[:], b_sb[:, k : k + 1], 0.0,
                        mybir.AluOpType.add, mybir.AluOpType.min,
                    )
                    pn = small.tile([C, 2], F32, tag="pn")
                    nc.vector.scalar_tensor_tensor(
                        pn[:], zmin[:], a_sb[:, k : k + 1], pos[:],
                        mybir.AluOpType.mult, mybir.AluOpType.add,
                    )
                    xn = small.tile([C, 2], F32, tag="xn")
                    nc.vector.tensor_add(xn[:], pn[:], xf)
                    xf = xn[:]

                # scores(b) = sigmoid(x_max + x_avg): (C, 1)
                ssum = small.tile([C, 1], F32, tag="ssum")
                nc.vector.tensor_add(ssum[:], xf[:, 0:1], xf[:, 1:2])
                scores = small.tile([C, 1], F32, tag="scores")
                nc.scalar.activation(
                    out=scores[:], in_=ssum[:],
                    func=mybir.ActivationFunctionType.Sigmoid,
                )
                # broadcast to all partitions on-chip: (C,1) -T-> (1,C), then
                # a K=1 ones matmul fans it out to (P, C); widen to a full
                # fp16 (P, KF, C) tile so the pass-2 muls see dense step-1
                # operands.
                sc_t = psum.tile([1, C], F32, tag="sc_t")
                nc.tensor.transpose(sc_t[:], scores[:], identity[:C, :C])
                sc_sb = small.tile([1, C], F32, tag="sc_sb")
                nc.vector.tensor_copy(sc_sb[:], sc_t[:])
                bc_ps = psum.tile([P, C], F32, tag="bc")
                nc.tensor.matmul(
                    bc_ps[:], ones_row[:], sc_sb[:], start=True, stop=True
                )
                bcb = small.tile([P, C], F16, tag="bcb")
                nc.vector.tensor_copy(bcb[:], bc_ps[:])
                sct = sctp.tile([P, KF, C], F16, tag="sct")
                nc.vector.tensor_copy(
                    sct[:], bcb[:].unsqueeze(1).to_broadcast([P, KF, C])
                )

                # ---- Pass 2(b): 16-bit multiply in place, ACT widens to fp32
                # staging, stores ride the HWDGE (SP) ring so the write stream
                # overlaps the next batch's SWDGE read stream.
                for t in range(T):
                    stg = stgp.tile([P, KF, C], F32, tag="stg")
                    if b == 0:
                        # early batch: DVE is contended (next batch's trees),
                        # so 2x-mode fp16 mul in place + widen on idle ACT
                        nc.vector.tensor_mul(
                            cached[t][:], cached[t][:], sct[:]
                        )
                        nc.scalar.copy(out=stg[:], in_=cached[t][:])
                    else:
                        # last batch: DVE is free, ACT latency hurts — fuse
                        # multiply+widen into one DVE op straight to staging
                        nc.vector.tensor_mul(stg[:], cached[t][:], sct[:])
                    nc.sync.dma_start(out=out_t[b, t], in_=stg[:])

    _split_dma_waits(nc)
    return nc


def _prep_inputs(features, W1, b1, a1):
    feats = np.ascontiguousarray(features, dtype=np.float32).reshape(B, HW, C)
    # lhsT layout: wT[c_in, k, c_out] = W1[k, c_out, c_in]
    wT = np.ascontiguousarray(np.transpose(np.asarray(W1, np.float32), (2, 0, 1)))
    bT = np.ascontiguousarray(np.asarray(b1, np.float32).T)            # (C, 7)
    aT = np.ascontiguousarray(
        np.broadcast_to(np.asarray(a1, np.float32), (C, CONV_NUM))
    )
    return feats, wT, bT, aT


def kernel(features, W1, b1, a1):
    global LAST_EXEC_NS
    feats, wT, bT, aT = _prep_inputs(features, W1, b1, a1)
    nc = _build_nc()
    in_maps = [
        {
            "features": feats[i * BPC : (i + 1) * BPC],
            "wT": wT,
            "bT": bT,
            "aT": aT,
        }
        for i in range(NCORES)
    ]
    import os

    res = run_bass_kernel_spmd(
        nc,
        in_maps,
        list(range(NCORES)),
        trace=PROFILE,
        tmpdir=os.environ.get("BASS_TMPDIR"),
    )
    global LAST_RESULTS
    LAST_RESULTS = res
    LAST_EXEC_NS = res.exec_time_ns
    out = np.concatenate(
        [res.results[i]["out"].reshape(BPC, H, W, C) for i in range(NCORES)], axis=0
    )
    return out
